# revision 14
# baseline (speedup 1.0000x reference)
"""2-layer GCN + classifier on 8 Trainium2 NeuronCores.

Strategy (graph/data parallel):
- Nodes sharded 8 ways by contiguous range (12500/core). Edges (excluding
  self-loops) partitioned by dst shard on host, grouped by (dst-tile of 128,
  src-chunk of 32768); per-(t,k) runs padded to the max count across cores
  and 128-aligned in the gather stream (SPMD shares one schedule).
- Per GCN layer: each core builds its shard of the gather table
  T = dinv * (Z @ W) (bf16) plus a TRANSPOSED dinv^2-scaled copy (the
  self-loop term), AllGather -> full table in every core's HBM.
  Aggregation: dma_gather rows by src (int16 chunk-local idx); segment-sum via
  PSUM-accumulated bf16 matmuls against HOST-PRECOMPUTED one-hot blocks that
  carry dinv[dst]. Self-loop term added from the transposed table during the
  epilogue (no gather for self-loops).
- Epilogue: x = agg + selfT; elu(x+b)+1 = relu(x+b) + min(exp(x+b),1) with
  relu/exp on the Scalar engine; the -1 is folded into the next matmul as a
  constant correction row (-colsum(W)) or into the classifier bias on host.
- Classifier + log_softmax per node tile on device; host concatenates shards.
"""
import sys

sys.path.insert(0, "/opt/trn_rl_repo")

import numpy as np
import ml_dtypes

import concourse.bacc as bacc
import concourse.tile as tile
from concourse import mybir
from concourse.bass_utils import run_bass_kernel_spmd

# ---------------- problem constants (hardcoded per task statement) ----------
N = 100000
E = 1600000
F_IN = 128
HID = 128
C_OUT = 40
NCORES = 8
NSH = N // NCORES          # 12500 nodes per core
P = 128
NT = (NSH + P - 1) // P    # 98 dst tiles per core (last has 84 rows)
NSH_PAD = NT * P           # 12544
CH = 32768                 # gather chunk rows (int16 idx limit)
NCHUNK = (N + CH - 1) // CH  # 4
TG = 12                    # dst tiles per tile-group (PSUM: 3 banks x 2 bufs)
NTG = (NT + TG - 1) // TG  # 9 tile groups

F32 = mybir.dt.float32
BF16 = mybir.dt.bfloat16
I16 = mybir.dt.int16


def _split_hi_lo(w):
    hi = w.astype(ml_dtypes.bfloat16)
    lo = (w - hi.astype(np.float32)).astype(ml_dtypes.bfloat16)
    return hi, lo


def _prep_host(x, edge_index, W0, b0, W1, b1, Wl, bl):
    """Shard + reorder edges; build all per-core device input arrays."""
    src = np.asarray(edge_index[0]).astype(np.int64)
    dst = np.asarray(edge_index[1]).astype(np.int64)
    # degree includes the self-loop; self-loops handled via transposed tables
    deg = (np.bincount(dst, minlength=N) + 1).astype(np.float32)
    dinv = (1.0 / np.sqrt(deg)).astype(np.float32)

    # ---- per-core edge grouping by (dst_tile, src_chunk) ----
    core_of = dst // NSH
    per_core = []
    cnt = np.zeros((NCORES, NT * NCHUNK), dtype=np.int64)
    for c in range(NCORES):
        sel = core_of == c
        es = src[sel]
        ed = dst[sel] - c * NSH
        tile_id = ed // P
        chunk_id = es // CH
        key = tile_id * NCHUNK + chunk_id
        order = np.argsort(key, kind="stable")
        es, ed, key = es[order], ed[order], key[order]
        cnt[c] = np.bincount(key, minlength=NT * NCHUNK)
        per_core.append((es, ed % P, ed, key))

    cnt_max = cnt.max(axis=0).reshape(NT, NCHUNK)  # padded length per (t,k)

    # ---- static schedule: per (g,k) streams and matmul pieces (base-0) ----
    L_gk = np.zeros((NTG, NCHUNK), dtype=np.int64)
    pieces = [[[] for _ in range(NCHUNK)] for _ in range(NTG)]
    run_start = np.zeros((NT, NCHUNK), dtype=np.int64)
    np_gk = np.zeros((NTG, NCHUNK), dtype=np.int64)
    for g in range(NTG):
        tiles = list(range(g * TG, min((g + 1) * TG, NT)))
        for k in range(NCHUNK):
            pos = 0
            pcol = 0
            for t in tiles:
                n = int(cnt_max[t][k])
                run_start[t][k] = pos
                s = pos
                while s < pos + n:
                    e = min((s // P + 1) * P, pos + n)
                    pieces[g][k].append((t, s // P, 0, ((e - 1) % P) + 1, pcol))
                    pcol += 1
                    s = e
                pos += -(-n // P) * P  # 128-aligned runs
            L_gk[g][k] = pos
            np_gk[g][k] = pcol

    tot_idx16 = int(L_gk.sum())
    tot_pieces = int(np_gk.sum())

    gidx_all = np.zeros((NCORES, 16, tot_idx16 // 16), dtype=np.int16)
    oh_all = np.zeros((NCORES, 128, tot_pieces * P), dtype=ml_dtypes.bfloat16)

    piecemap = {}
    for g in range(NTG):
        for k in range(NCHUNK):
            pm = np.full(int(L_gk[g][k]), -1, dtype=np.int64)
            for (t, blk, a, b, pcol) in pieces[g][k]:
                pm[blk * P + a : blk * P + b] = pcol
            piecemap[(g, k)] = pm

    for c in range(NCORES):
        es, slots, ed_full, key = per_core[c]
        starts = np.zeros(NT * NCHUNK + 1, dtype=np.int64)
        np.cumsum(cnt[c], out=starts[1:])
        goff16 = 0
        pcoloff = 0
        for g in range(NTG):
            tiles = list(range(g * TG, min((g + 1) * TG, NT)))
            for k in range(NCHUNK):
                Lg = int(L_gk[g][k])
                pm = piecemap[(g, k)]
                stream_idx = np.zeros(Lg, dtype=np.int16)
                pos_l = []
                cols_l = []
                vals_l = []
                for t in tiles:
                    a0, b0_ = starts[t * NCHUNK + k], starts[t * NCHUNK + k + 1]
                    n_real = int(b0_ - a0)
                    pos = int(run_start[t][k])
                    stream_idx[pos : pos + n_real] = (es[a0:b0_] - k * CH).astype(
                        np.int16)
                    pos_l.append(pos + np.arange(n_real, dtype=np.int64))
                    cols_l.append(slots[a0:b0_])
                    vals_l.append(dinv[ed_full[a0:b0_] + c * NSH])
                if pos_l:
                    p_abs = np.concatenate(pos_l)
                    scol = np.concatenate(cols_l)
                    sval = np.concatenate(vals_l)
                    pcols = pcoloff + pm[p_abs]
                    oh_all[c][p_abs % P, pcols * P + scol] = sval.astype(
                        ml_dtypes.bfloat16)
                gidx_all[c][:, goff16 : goff16 + Lg // 16] = stream_idx.reshape(
                    -1, 16).T
                goff16 += Lg // 16
                pcoloff += int(np_gk[g][k])

    gidx_rep = np.tile(gidx_all, (1, 8, 1))

    # degree layouts: column (per-node dinv for tables), row (1/deg for selfT)
    deg_col = np.ones((NCORES, 128, NT), dtype=np.float32)
    deg_row = np.ones((NCORES, 1, NSH_PAD), dtype=np.float32)
    for c in range(NCORES):
        d = deg[c * NSH : (c + 1) * NSH]
        dp = np.concatenate([d, np.ones(NSH_PAD - NSH, dtype=np.float32)])
        deg_col[c] = dp.reshape(NT, P).T
        deg_row[c, 0] = dp

    xT_hi = np.zeros((NCORES, 128, NSH_PAD), dtype=ml_dtypes.bfloat16)
    xT_lo = np.zeros((NCORES, 128, NSH_PAD), dtype=ml_dtypes.bfloat16)
    for c in range(NCORES):
        xs = np.asarray(x[c * NSH : (c + 1) * NSH]).astype(np.float32).T
        hi, lo = _split_hi_lo(xs)
        xT_hi[c, :, :NSH] = hi
        xT_lo[c, :, :NSH] = lo

    W0f = np.asarray(W0, dtype=np.float32)
    W1f = np.asarray(W1, dtype=np.float32)
    Wlf = np.asarray(Wl, dtype=np.float32)
    W0h, W0l = _split_hi_lo(W0f)
    W1h, W1l = _split_hi_lo(W1f)
    Wlh, Wll = _split_hi_lo(Wlf)
    b0c = np.asarray(b0, dtype=np.float32).reshape(128, 1)
    b1c = np.asarray(b1, dtype=np.float32).reshape(128, 1)
    blb = np.tile(
        (np.asarray(bl, dtype=np.float32) - Wlf.sum(axis=0)).reshape(1, C_OUT),
        (128, 1))
    c1 = -W1f.sum(axis=0)
    c1h, c1l = _split_hi_lo(c1)
    corr1 = np.zeros((2, HID), dtype=ml_dtypes.bfloat16)
    corr1[0] = c1h
    corr1[1] = c1l
    ones2 = np.ones((2, 128), dtype=ml_dtypes.bfloat16)
    c1col = c1.reshape(128, 1).astype(np.float32)

    in_maps = []
    for c in range(NCORES):
        in_maps.append(
            {
                "xT_hi": xT_hi[c],
                "xT_lo": xT_lo[c],
                "gidx": gidx_rep[c],
                "oh": oh_all[c],
                "deg_col": deg_col[c],
                "deg_row": deg_row[c],
                "W0h": W0h, "W0l": W0l,
                "W1h": W1h, "W1l": W1l,
                "Wlh": Wlh, "Wll": Wll,
                "b0c": b0c, "b1c": b1c, "blb": blb,
                "corr1": corr1, "ones2": ones2, "c1col": c1col,
            }
        )
    sched = (tuple(tuple(int(L_gk[g][k]) for k in range(NCHUNK)) for g in range(NTG)),
             tuple(tuple(tuple(p) for p in sum([pieces[g][k] for k in range(NCHUNK)], []))
                   for g in range(NTG)))
    return in_maps, L_gk, pieces, np_gk, tot_idx16, tot_pieces, sched


def _build_program(L_gk, pieces, np_gk, tot_idx16, tot_pieces):
    nc = bacc.Bacc(num_devices=NCORES)
    xT_hi = nc.declare_dram_parameter("xT_hi", [128, NSH_PAD], BF16, isOutput=False)
    xT_lo = nc.declare_dram_parameter("xT_lo", [128, NSH_PAD], BF16, isOutput=False)
    gidx = nc.declare_dram_parameter("gidx", [128, tot_idx16 // 16], I16, isOutput=False)
    oh_ext = nc.declare_dram_parameter("oh", [128, tot_pieces * P], BF16, isOutput=False)
    deg_col = nc.declare_dram_parameter("deg_col", [128, NT], F32, isOutput=False)
    deg_row = nc.declare_dram_parameter("deg_row", [1, NSH_PAD], F32, isOutput=False)
    W0h = nc.declare_dram_parameter("W0h", [128, HID], BF16, isOutput=False)
    W0l = nc.declare_dram_parameter("W0l", [128, HID], BF16, isOutput=False)
    W1h = nc.declare_dram_parameter("W1h", [128, HID], BF16, isOutput=False)
    W1l = nc.declare_dram_parameter("W1l", [128, HID], BF16, isOutput=False)
    Wlh = nc.declare_dram_parameter("Wlh", [128, C_OUT], BF16, isOutput=False)
    Wll = nc.declare_dram_parameter("Wll", [128, C_OUT], BF16, isOutput=False)
    b0c = nc.declare_dram_parameter("b0c", [128, 1], F32, isOutput=False)
    b1c = nc.declare_dram_parameter("b1c", [128, 1], F32, isOutput=False)
    blb = nc.declare_dram_parameter("blb", [128, C_OUT], F32, isOutput=False)
    corr1 = nc.declare_dram_parameter("corr1", [2, HID], BF16, isOutput=False)
    ones2 = nc.declare_dram_parameter("ones2", [2, 128], BF16, isOutput=False)
    c1col = nc.declare_dram_parameter("c1col", [128, 1], F32, isOutput=False)
    out_ext = nc.declare_dram_parameter("out", [NSH, C_OUT], F32, isOutput=True)

    t1_shard = nc.dram_tensor("t1_shard", [NSH, HID], BF16)
    t2_shard = nc.dram_tensor("t2_shard", [NSH, HID], BF16)
    sfT1 = nc.dram_tensor("sfT1", [128, NSH_PAD], BF16)
    sfT2 = nc.dram_tensor("sfT2", [128, NSH_PAD], BF16)
    T1_full = nc.dram_tensor("T1_full", [N, HID], BF16, addr_space="Shared")
    T2_full = nc.dram_tensor("T2_full", [N, HID], BF16, addr_space="Shared")

    max_nb = max(int(-(-L_gk[g][k] // P)) for g in range(NTG) for k in range(NCHUNK))
    max_np = int(np_gk.max())
    max_i16 = max(int(-(-L_gk[g][k] // 16)) for g in range(NTG) for k in range(NCHUNK))

    from contextlib import ExitStack
    with tile.TileContext(nc) as tc, ExitStack() as es:
        cpool = es.enter_context(tc.tile_pool(name="const", bufs=1))
        xpool = es.enter_context(tc.tile_pool(name="xp", bufs=2))
        gpool = es.enter_context(tc.tile_pool(name="gp", bufs=3))
        hpool = es.enter_context(tc.tile_pool(name="hp", bufs=3))
        ipool = es.enter_context(tc.tile_pool(name="ip", bufs=3))
        spool = es.enter_context(tc.tile_pool(name="sf", bufs=3))
        zpool = es.enter_context(tc.tile_pool(name="zp", bufs=6))
        opool = es.enter_context(tc.tile_pool(name="op", bufs=2))
        apsum = es.enter_context(tc.tile_pool(name="apsum", bufs=2, space="PSUM"))
        wpsum = es.enter_context(tc.tile_pool(name="wpsum", bufs=2, space="PSUM"))

        # ---- constants ----
        w0h_t = cpool.tile([128, HID], BF16, tag="w0h")
        w0l_t = cpool.tile([128, HID], BF16, tag="w0l")
        w1h_t = cpool.tile([128, HID], BF16, tag="w1h")
        w1l_t = cpool.tile([128, HID], BF16, tag="w1l")
        wlh_t = cpool.tile([128, C_OUT], BF16, tag="wlh")
        wll_t = cpool.tile([128, C_OUT], BF16, tag="wll")
        b0_t = cpool.tile([128, 1], F32, tag="b0")
        b1_t = cpool.tile([128, 1], F32, tag="b1")
        blb_t = cpool.tile([128, C_OUT], F32, tag="blb")
        corr1_t = cpool.tile([2, HID], BF16, tag="corr1")
        ones2_t = cpool.tile([2, 128], BF16, tag="ones2")
        c1col_t = cpool.tile([128, 1], F32, tag="c1col")
        for tt, ext in [(w0h_t, W0h), (w0l_t, W0l), (w1h_t, W1h), (w1l_t, W1l),
                        (wlh_t, Wlh), (wll_t, Wll), (b0_t, b0c), (b1_t, b1c),
                        (blb_t, blb), (corr1_t, corr1), (ones2_t, ones2),
                        (c1col_t, c1col)]:
            nc.sync.dma_start(out=tt[:], in_=ext[:, :])

        # ---- dinv column layout (for table scaling) ----
        dcol_raw = cpool.tile([128, NT], F32, tag="dcolr")
        nc.sync.dma_start(out=dcol_raw[:], in_=deg_col[:, :])
        dcol_s = cpool.tile([128, NT], F32, tag="dcols")
        nc.scalar.activation(dcol_s[:], dcol_raw[:], mybir.ActivationFunctionType.Sqrt)
        dinv_col = cpool.tile([128, NT], F32, tag="dcol")
        nc.vector.reciprocal(dinv_col[:], dcol_s[:])

        # ---- 1/deg broadcast tile (selfT scale) ----
        d2b = cpool.tile([128, NSH_PAD], F32, tag="d2b")
        nc.sync.dma_start(out=d2b[:1, :], in_=deg_row[:, :])
        nc.vector.reciprocal(d2b[:1, :], d2b[:1, :])
        rows_done = 1
        while rows_done < 128:
            n = min(rows_done, 128 - rows_done)
            nc.sync.dma_start(out=d2b[rows_done : rows_done + n, :],
                              in_=d2b[:n, :])
            rows_done += n

        # ---- phase 1: T1 shard = dinv * (X @ W0); sfT1 = (X @ W0)^T / deg --
        XS = 8
        for s0 in range(0, NT, XS):
            ntile = min(XS, NT - s0)
            w = ntile * P
            xh = xpool.tile([128, XS * P], BF16, tag="xh")
            xl = xpool.tile([128, XS * P], BF16, tag="xl")
            nc.sync.dma_start(out=xh[:, :w], in_=xT_hi[:, s0 * P : s0 * P + w])
            nc.sync.dma_start(out=xl[:, :w], in_=xT_lo[:, s0 * P : s0 * P + w])
            for j in range(ntile):
                t = s0 + j
                rows = min(P, NSH - t * P)
                xhj = xh[:, j * P : (j + 1) * P]
                xlj = xl[:, j * P : (j + 1) * P]
                bank = wpsum.tile([P, 512], F32, tag="wps", space="PSUM")
                ps = bank[:, :HID]
                nc.tensor.matmul(out=ps, lhsT=xhj, rhs=w0h_t[:], start=True, stop=False,
                                 skip_group_check=True)
                nc.tensor.matmul(out=ps, lhsT=xhj, rhs=w0l_t[:], start=False, stop=False,
                                 skip_group_check=True)
                nc.tensor.matmul(out=ps, lhsT=xlj, rhs=w0h_t[:], start=False, stop=True,
                                 skip_group_check=True)
                tb = opool.tile([P, HID], BF16, tag="tb")
                nc.vector.tensor_scalar(out=tb[:], in0=ps,
                                        scalar1=dinv_col[:, t : t + 1],
                                        scalar2=None, op0=mybir.AluOpType.mult)
                nc.sync.dma_start(out=t1_shard[t * P : t * P + rows, :],
                                  in_=tb[:rows, :])
                # transposed: psT = (X@W0)^T for this tile's columns
                psT = bank[:, 128:256]
                nc.tensor.matmul(out=psT, lhsT=w0h_t[:], rhs=xhj, start=True, stop=False,
                                 skip_group_check=True)
                nc.tensor.matmul(out=psT, lhsT=w0l_t[:], rhs=xhj, start=False, stop=False,
                                 skip_group_check=True)
                nc.tensor.matmul(out=psT, lhsT=w0h_t[:], rhs=xlj, start=False, stop=True,
                                 skip_group_check=True)
                sfb = opool.tile([P, P], BF16, tag="sfb")
                nc.vector.tensor_tensor(out=sfb[:], in0=psT,
                                        in1=d2b[:, t * P : (t + 1) * P],
                                        op=mybir.AluOpType.mult)
                nc.sync.dma_start(out=sfT1[:, t * P : (t + 1) * P], in_=sfb[:])

        # ---- allgather T1 ----
        nc.gpsimd.collective_compute(
            "AllGather", mybir.AluOpType.bypass,
            replica_groups=[list(range(NCORES))],
            ins=[t1_shard[:].opt()], outs=[T1_full[:].opt()],
        )

        # ---- aggregation layers ----
        def agg_layer(T_full, sfT, layer):
            goff16 = [0]
            pcoloff = [0]
            for g in range(NTG):
                tiles = list(range(g * TG, min((g + 1) * TG, NT)))
                all_pieces = []
                for k in range(NCHUNK):
                    for (t, blk, a, b, pcol) in pieces[g][k]:
                        all_pieces.append((k, t, blk, a, b, pcol))
                first_of = {}
                last_of = {}
                for i, (k, t, blk, a, b, pcol) in enumerate(all_pieces):
                    if t not in first_of:
                        first_of[t] = (k, pcol)
                    last_of[t] = (k, pcol)

                nbank = (len(tiles) + 3) // 4
                banks = [apsum.tile([P, 512], F32, tag=f"agg{i}", space="PSUM",
                                    name=f"aggbank{i}")
                         for i in range(nbank)]

                def agg_ap(ti):
                    i = tiles.index(ti)
                    return banks[i // 4][:, (i % 4) * P : (i % 4 + 1) * P]

                for k in range(NCHUNK):
                    Lg = int(L_gk[g][k])
                    if Lg == 0:
                        continue
                    npg = int(np_gk[g][k])
                    nb = Lg // P
                    kend = min((k + 1) * CH, N)
                    idxt = ipool.tile([128, max_i16], I16, tag="gidx")
                    nc.sync.dma_start(out=idxt[:, : Lg // 16],
                                      in_=gidx[:, goff16[0] : goff16[0] + Lg // 16])
                    oht = hpool.tile([128, max_np * P], BF16, tag="oh")
                    nc.sync.dma_start(
                        out=oht[:, : npg * P],
                        in_=oh_ext[:, pcoloff[0] * P : (pcoloff[0] + npg) * P])
                    gbuf = gpool.tile([P, max_nb, P], BF16, tag="gath")
                    nc.gpsimd.dma_gather(
                        gbuf[:, :nb, :], T_full[k * CH : kend, :],
                        idxt[:, : Lg // 16], Lg, Lg, HID,
                        single_packet=False,
                    )
                    goff16[0] += Lg // 16
                    for (t, blk, a, b, pcol) in pieces[g][k]:
                        nc.tensor.matmul(
                            out=agg_ap(t),
                            lhsT=gbuf[a:b, blk, :],
                            rhs=oht[a:b, pcol * P : (pcol + 1) * P],
                            start=(first_of[t] == (k, pcol)),
                            stop=(last_of[t] == (k, pcol)),
                            skip_group_check=True,
                        )
                    pcoloff[0] += npg

                # ---- epilogue per tile ----
                for t in tiles:
                    rows = min(P, NSH - t * P)
                    bias = b0_t if layer == 1 else b1_t
                    sft = spool.tile([P, P], BF16, tag="sft")
                    nc.sync.dma_start(out=sft[:], in_=sfT[:, t * P : (t + 1) * P])
                    x = zpool.tile([P, P], F32, tag="x")
                    nc.vector.tensor_tensor(out=x[:], in0=agg_ap(t), in1=sft[:],
                                            op=mybir.AluOpType.add)
                    pmax = zpool.tile([P, P], F32, tag="pmax")
                    nc.scalar.activation(pmax[:], x[:],
                                         mybir.ActivationFunctionType.Relu,
                                         bias=bias[:])
                    e2 = zpool.tile([P, P], F32, tag="e2")
                    nc.scalar.activation(e2[:], x[:],
                                         mybir.ActivationFunctionType.Exp,
                                         bias=bias[:])
                    zbt = zpool.tile([P, P], F32, tag="zbt")
                    nc.vector.tensor_scalar(out=zbt[:], in0=e2[:], scalar1=1.0,
                                            scalar2=None, op0=mybir.AluOpType.min)
                    zb = zpool.tile([P, P], BF16, tag="zb")
                    nc.vector.tensor_tensor(out=zb[:], in0=zbt[:], in1=pmax[:],
                                            op=mybir.AluOpType.add)
                    if layer == 1:
                        bank = wpsum.tile([P, 512], F32, tag="wps", space="PSUM")
                        ps2 = bank[:, :HID]
                        nc.tensor.matmul(out=ps2, lhsT=zb[:], rhs=w1h_t[:],
                                         start=True, stop=False, skip_group_check=True)
                        nc.tensor.matmul(out=ps2, lhsT=zb[:], rhs=w1l_t[:],
                                         start=False, stop=False, skip_group_check=True)
                        nc.tensor.matmul(out=ps2, lhsT=ones2_t[:], rhs=corr1_t[:],
                                         start=False, stop=True, skip_group_check=True)
                        t2b = opool.tile([P, HID], BF16, tag="tb")
                        nc.vector.tensor_scalar(out=t2b[:], in0=ps2,
                                                scalar1=dinv_col[:, t : t + 1],
                                                scalar2=None, op0=mybir.AluOpType.mult)
                        nc.sync.dma_start(out=t2_shard[t * P : t * P + rows, :],
                                          in_=t2b[:rows, :])
                        # transposed table for layer-2 self term
                        psT2 = bank[:, 128:256]
                        nc.tensor.matmul(out=psT2, lhsT=w1h_t[:], rhs=zb[:],
                                         start=True, stop=False, skip_group_check=True)
                        nc.tensor.matmul(out=psT2, lhsT=w1l_t[:], rhs=zb[:],
                                         start=False, stop=True, skip_group_check=True)
                        sc = zpool.tile([P, P], F32, tag="sc")
                        nc.vector.tensor_scalar(out=sc[:], in0=psT2,
                                                scalar1=c1col_t[:], scalar2=None,
                                                op0=mybir.AluOpType.add)
                        sfb2 = opool.tile([P, P], BF16, tag="sfb")
                        nc.vector.tensor_tensor(out=sfb2[:], in0=sc[:],
                                                in1=d2b[:, t * P : (t + 1) * P],
                                                op=mybir.AluOpType.mult)
                        nc.sync.dma_start(out=sfT2[:, t * P : (t + 1) * P],
                                          in_=sfb2[:])
                    else:
                        bank = wpsum.tile([P, 512], F32, tag="wps", space="PSUM")
                        ps3 = bank[:, :C_OUT]
                        nc.tensor.matmul(out=ps3, lhsT=zb[:], rhs=wlh_t[:],
                                         start=True, stop=False, skip_group_check=True)
                        nc.tensor.matmul(out=ps3, lhsT=zb[:], rhs=wll_t[:],
                                         start=False, stop=True, skip_group_check=True)
                        lg = opool.tile([P, C_OUT], F32, tag="lg")
                        nc.vector.tensor_tensor(out=lg[:], in0=ps3,
                                                in1=blb_t[:], op=mybir.AluOpType.add)
                        mx = opool.tile([P, 1], F32, tag="mx")
                        nc.vector.tensor_reduce(out=mx[:], in_=lg[:],
                                                axis=mybir.AxisListType.X,
                                                op=mybir.AluOpType.max)
                        sh = opool.tile([P, C_OUT], F32, tag="sh")
                        nc.vector.tensor_scalar(out=sh[:], in0=lg[:], scalar1=mx[:],
                                                scalar2=None,
                                                op0=mybir.AluOpType.subtract)
                        ex = opool.tile([P, C_OUT], F32, tag="ex")
                        sm = opool.tile([P, 1], F32, tag="sm")
                        nc.scalar.activation(ex[:], sh[:],
                                             mybir.ActivationFunctionType.Exp,
                                             accum_out=sm[:])
                        ln = opool.tile([P, 1], F32, tag="ln")
                        nc.scalar.activation(ln[:], sm[:],
                                             mybir.ActivationFunctionType.Ln)
                        res = opool.tile([P, C_OUT], F32, tag="res")
                        nc.vector.tensor_scalar(out=res[:], in0=sh[:], scalar1=ln[:],
                                                scalar2=None,
                                                op0=mybir.AluOpType.subtract)
                        nc.sync.dma_start(out=out_ext[t * P : t * P + rows, :],
                                          in_=res[:rows, :])

        agg_layer(T1_full, sfT1, 1)
        nc.gpsimd.collective_compute(
            "AllGather", mybir.AluOpType.bypass,
            replica_groups=[list(range(NCORES))],
            ins=[t2_shard[:].opt()], outs=[T2_full[:].opt()],
        )
        agg_layer(T2_full, sfT2, 2)

    nc.finalize()
    return nc


_CACHE = {}


def kernel(**inputs):
    in_maps, L_gk, pieces, np_gk, tot_idx16, tot_pieces, sched = _prep_host(
        inputs["x"], inputs["edge_index"], inputs["W0"], inputs["b0"],
        inputs["W1"], inputs["b1"], inputs["Wl"], inputs["bl"])
    key = (tot_idx16, tot_pieces, sched)
    if key not in _CACHE:
        _CACHE[key] = _build_program(L_gk, pieces, np_gk, tot_idx16, tot_pieces)
    nc = _CACHE[key]
    trace = bool(int(__import__("os").environ.get("KERNEL_TRACE", "0")))
    res = run_bass_kernel_spmd(nc, in_maps, list(range(NCORES)), trace=trace)
    kernel.last_results = res
    out = np.concatenate([res.results[c]["out"] for c in range(NCORES)], axis=0)
    return out.astype(np.float32)
